# revision 51
# baseline (speedup 1.0000x reference)
"""Fused LayerNorm + single-head self-attention kernel for Trainium2 (8 NeuronCores).

Problem: x[4,64,64,128] -> LN(ch) -> QKV proj -> softmax(QK^T/sqrt(C)) V -> out proj.

Sharding: 2 cores per batch element. Each core computes its batch's full K/V
(4096 tokens) and one half of the queries (2048 rows). The host rotates each
core's batch so its query half leads (attention is invariant to k/v token
order), so queries are just x token-tiles 0..15 and the tokens are only
LayerNorm'd once; the SPMD program is uniform and needs no collectives.

Host folds gamma/beta and the 1/sqrt(C) softmax scale into the projection
weights, so the device LN is just (x-mu)*rstd.

The kernel is ACT-bound: 64 softmax-exp instructions of [128,1024] are ~73us
of Activation-engine time; everything else is scheduled around keeping that
stream gapless (the exp stream runs with zero gaps from ~10us to ~84us):
  - ACT runs (almost) nothing but exp; LN applies run on Pool (gpsimd), LN
    stats on DVE; rstd (one Ln+Exp pair per 8-tile group) is emitted several
    score-tiles ahead of the group's applies so it sits early in ACT's
    queue. PSUM evacuations are DVE-only (Pool cannot access PSUM), except
    during startup/drain when the otherwise-idle ACT takes them.
  - no DMA ever uses the ACT queue (a config occupies the ACT sequencer for
    ~670ns); loads/stores spread over the SP and Pool(SWDGE) queues, with
    the critical x group-0 load ahead of the weights in HWDGE order.
  - scores are emitted one kv-half behind kv production, so the kT tiles
    they read were evacuated ~4.5us earlier and the per-half
    PE->DVE->PE chain never gates the exp stream. kp lives in a separate
    PSUM pool so the two score PSUM bufs only ever rotate scores.
  - attn@v for each query subtile j is accumulated in pieces via PSUM->SBUF
    adds on DVE; at most one piece lands per exp window, placed where its
    exp inputs are complete before PE reaches it.
  - v carries a ones column so attn@v also yields softmax denominators;
    normalization happens after the Wo projection (row scale commutes).
  - drain: all final pieces are emitted before all output chains (PE never
    stalls in-order on a chain), ACT evacuates the chains' PSUM, outputs
    pair up into one DMA per two chains, split across HWDGE and SWDGE.
"""

import os
import sys
from contextlib import ExitStack

import numpy as np

for _p in ("/opt/trn_rl_repo", "/root/.axon_site/_ro/trn_rl_repo"):
    if os.path.isdir(_p) and _p not in sys.path:
        sys.path.insert(0, _p)

import concourse.bass as bass
import concourse.tile as tile
from concourse import bacc, mybir
from concourse.bass import ds, ts
from concourse._compat import with_exitstack
from concourse.bass_utils import run_bass_kernel_spmd

B, HH, WW, C = 4, 64, 64, 128
S = HH * WW  # 4096 tokens per batch
SQ = S // 2  # 2048 query rows per core
P = 128
NT = S // P  # 32 kv token tiles
QBLK = 1024
NBLK = SQ // QBLK  # 2 query blocks per core
NJ = QBLK // P  # 8 query subtiles per block
EPS = 1e-5

F32 = mybir.dt.float32
BF16 = mybir.dt.bfloat16


@with_exitstack
def _attention_kernel(ctx: ExitStack, tc: tile.TileContext, aps: dict):
    nc = tc.nc
    x, out = aps["x"], aps["out"]

    consts = ctx.enter_context(tc.tile_pool(name="consts", bufs=1))
    bigp = ctx.enter_context(tc.tile_pool(name="big", bufs=1))
    lnp = ctx.enter_context(tc.tile_pool(name="ln", bufs=3))
    statp = ctx.enter_context(tc.tile_pool(name="stat", bufs=3))
    nxp = ctx.enter_context(tc.tile_pool(name="nx", bufs=6))
    expp = ctx.enter_context(tc.tile_pool(name="expp", bufs=2))
    aop = ctx.enter_context(tc.tile_pool(name="aop", bufs=9))
    smallp = ctx.enter_context(tc.tile_pool(name="smallp", bufs=8))
    outp = ctx.enter_context(tc.tile_pool(name="outp", bufs=6))
    sap = ctx.enter_context(tc.tile_pool(name="sap", bufs=18))
    # PSUM plan (8 banks):
    #   u1: [128,1024] x2 bufs = 4 banks  (k/q projections + scores)
    #   u2: [128,4,128] x2 bufs = 2 banks (transposes, v proj, out proj)
    #   u3: [128,129] x2 bufs = 2 banks   (attnv piece accumulators)
    u1 = ctx.enter_context(tc.tile_pool(name="u1", bufs=2, space="PSUM"))
    u2 = ctx.enter_context(tc.tile_pool(name="u2", bufs=2, space="PSUM"))
    u3 = ctx.enter_context(tc.tile_pool(name="u3", bufs=2, space="PSUM"))

    def group_dma(src_t, g, qa, qb):
        xg = lnp.tile([P, 8, C], F32, tag="xg")
        for h, eng in ((0, qa), (1, qb)):
            eng.dma_start(
                out=xg[:, 4 * h:4 * h + 4, :],
                in_=src_t[(g * 8 + 4 * h) * P:(g * 8 + 4 * h + 4) * P, :]
                .rearrange("(i p) c -> p i c", p=P),
            )
        return xg

    # The x group-0 load issues FIRST so its HWDGE descriptor-gen slots
    # (~630ns each, serialized) precede the weight loads. No DMA ever uses
    # the ACT queue: each config would occupy the ACT sequencer for ~670ns
    # and stall the exp stream.
    xg_x0 = group_dma(x, 0, nc.sync, nc.gpsimd)

    idf = consts.tile([P, P], F32, tag="idf")
    nc.sync.dma_start(out=idf, in_=aps["ident"])
    id_b = consts.tile([P, P], BF16, tag="idb")
    nc.gpsimd.tensor_copy(id_b, idf)
    w_b = {}
    for name in ("wk", "wq"):
        wf = consts.tile([C, C], F32, tag=f"{name}_f")
        nc.sync.dma_start(out=wf, in_=aps[name])
        wb = consts.tile([C, C], BF16, tag=f"{name}_b")
        nc.gpsimd.tensor_copy(wb, wf)
        w_b[name] = wb
    bq_s = consts.tile([C, 1], F32, tag="bq")
    nc.sync.dma_start(out=bq_s, in_=aps["bq"])
    eps_t = consts.tile([P, 1], F32, tag="eps")
    nc.vector.memset(eps_t, EPS)

    def late_consts():
        for name in ("wv", "wo"):
            wf = consts.tile([C, C], F32, tag=f"{name}_f")
            nc.sync.dma_start(out=wf, in_=aps[name])
            wb = consts.tile([C, C], BF16, tag=f"{name}_b")
            nc.gpsimd.tensor_copy(wb, wf)
            w_b[name] = wb
        bob = consts.tile([P, C], F32, tag="bob")
        nc.sync.dma_start(out=bob, in_=aps["bob"])
        return bob

    # --- big persistent SBUF tensors
    nxT = bigp.tile([P, S], BF16, tag="nxT")      # normalized x, transposed
    kT = bigp.tile([P, S], BF16, tag="kT")
    qT = bigp.tile([P, SQ], BF16, tag="qT")
    vsb = bigp.tile([P, NT, 130], BF16, tag="vsb")  # [tok, c] + ones col at 128
    nc.vector.memset(vsb[:, :, 128:129], 1.0)
    eTs = []
    for _bi in range(NBLK):
        eT_blk = expp.tile([P, NT, QBLK], BF16, tag="eT")
        eTs.append(eT_blk)

    def emit_rstd(rstd, mv, sl):
        # rstd = exp(-0.5*ln(var+eps)); Ln and Exp share one activation
        # table set so this never reloads tables mid-stream
        nc.scalar.activation(
            rstd[:, sl], mv[:, sl, 1],
            func=mybir.ActivationFunctionType.Ln,
            bias=eps_t, scale=1.0)
        nc.scalar.activation(
            rstd[:, sl], rstd[:, sl],
            func=mybir.ActivationFunctionType.Exp,
            scale=-0.5)

    def ln_load(src, g, qa, qb):
        # DMA one 8-tile group, LN stats (DVE), rstd (ACT). Emitted several
        # score-tiles ahead of ln_apply so the rstd pair sits early in the
        # ACT queue and the applies never wait on it.
        xg = lnp.tile([P, 8, C], F32, tag="xg")
        for h, eng in ((0, qa), (1, qb)):
            eng.dma_start(
                out=xg[:, 4 * h:4 * h + 4, :],
                in_=src[(g * 8 + 4 * h) * P:(g * 8 + 4 * h + 4) * P, :]
                .rearrange("(i p) c -> p i c", p=P),
            )
        st = statp.tile([P, 8, 6], F32, tag="st")
        mv = statp.tile([P, 8, 2], F32, tag="mv")
        rstd = statp.tile([P, 8], F32, tag="rstd")
        for i in range(8):
            nc.vector.bn_stats(st[:, i, :], xg[:, i, :])
            nc.vector.bn_aggr(mv[:, i, :], st[:, i, :])
        emit_rstd(rstd, mv, slice(0, 8))
        return xg, mv, rstd

    def ln_apply(state, g, dstT, half_hook=None):
        # normalize (Pool), PE-transpose into dstT columns, DVE evacuation
        xg, mv, rstd = state
        for half in range(2):
            tp = u2.tile([P, 4, P], F32, tag="u2")
            for i in range(4 * half, 4 * half + 4):
                nxt = nxp.tile([P, C], BF16, tag="nxt")
                nc.gpsimd.tensor_scalar(
                    nxt, xg[:, i, :], mv[:, i, 0:1], rstd[:, i:i + 1],
                    mybir.AluOpType.subtract, mybir.AluOpType.mult)
                nc.tensor.matmul(tp[:, i % 4, :], lhsT=nxt, rhs=id_b,
                                 start=True, stop=True)
            base = (g * 8 + 4 * half) * P
            nc.vector.tensor_copy(dstT[:, ds(base, 4 * P)], tp)
            if half_hook is not None:
                half_hook(half)

    def ln_group0(xg, g, dstT, half_hook=None):
        # startup variant: stats + rstd + apply per 4-tile half for the
        # shortest chain to the first scores; PSUM evacuations go to ACT,
        # which is idle until the first exp
        st = statp.tile([P, 8, 6], F32, tag="st")
        mv = statp.tile([P, 8, 2], F32, tag="mv")
        rstd = statp.tile([P, 8], F32, tag="rstd")
        for half in range(2):
            sl = slice(4 * half, 4 * half + 4)
            for i in range(4 * half, 4 * half + 4):
                nc.vector.bn_stats(st[:, i, :], xg[:, i, :])
                nc.vector.bn_aggr(mv[:, i, :], st[:, i, :])
            emit_rstd(rstd, mv, sl)
            tp = u2.tile([P, 4, P], F32, tag="u2")
            for i in range(4 * half, 4 * half + 4):
                nxt = nxp.tile([P, C], BF16, tag="nxt")
                nc.gpsimd.tensor_scalar(
                    nxt, xg[:, i, :], mv[:, i, 0:1], rstd[:, i:i + 1],
                    mybir.AluOpType.subtract, mybir.AluOpType.mult)
                nc.tensor.matmul(tp[:, i % 4, :], lhsT=nxt, rhs=id_b,
                                 start=True, stop=True)
            base = (g * 8 + 4 * half) * P
            nc.scalar.copy(dstT[:, ds(base, 4 * P)], tp)
            if half_hook is not None:
                half_hook(half)

    def emit_qproj_half(j, h, on_act=False):
        # half-granular: the first scores only wait on 512 columns of qT,
        # and the bias/copy stays small wherever it lands
        qp = u3.tile([P, 512], F32, tag="u3")
        nc.tensor.matmul(qp, lhsT=w_b["wq"],
                         rhs=nxT[:, ds(j * QBLK + h * 512, 512)],
                         start=True, stop=True)
        dst = qT[:, ds(j * QBLK + h * 512, 512)]
        if on_act:
            nc.scalar.activation(
                dst, qp, func=mybir.ActivationFunctionType.Identity,
                bias=bq_s)
        else:
            nc.vector.tensor_scalar(
                dst, qp, bq_s, None, mybir.AluOpType.add)

    def emit_scores(b, i):
        sp = u1.tile([P, QBLK], F32, tag="u1")
        for h in range(2):
            nc.tensor.matmul(sp[:, ts(h, 512)], lhsT=kT[:, ts(i, P)],
                             rhs=qT[:, ds(b * QBLK + h * 512, 512)],
                             start=True, stop=True)
        nc.scalar.activation(eTs[b][:, i, :], sp,
                             func=mybir.ActivationFunctionType.Exp)

    def v_half(g, half):
        base = g * 8 + 4 * half
        vp = u2.tile([P, 4, C], F32, tag="u2")
        for i in range(4):
            nc.tensor.matmul(vp[:, i, :], lhsT=nxT[:, ts(base + i, P)],
                             rhs=w_b["wv"], start=True, stop=True)
        nc.vector.tensor_copy(vsb[:, ds(base, 4), 0:128], vp)

    def kv_half(g, half, with_v=True, on_act=False):
        # kp lives in u3, not u1: sharing the two score bufs would insert a
        # non-score consumer into the exp-paced rotation and hiccup ACT
        # once per half-group
        base = g * 8 + 4 * half
        kp = u3.tile([P, 512], F32, tag="u3")
        nc.tensor.matmul(kp, lhsT=w_b["wk"], rhs=nxT[:, ds(base * P, 512)],
                         start=True, stop=True)
        if on_act:
            nc.scalar.copy(kT[:, ds(base * P, 512)], kp)
        else:
            nc.vector.tensor_copy(kT[:, ds(base * P, 512)], kp)
        if with_v:
            v_half(g, half)

    # attn@v piece: accumulate kv tiles [t0,t1) for query subtile j of block
    # b into PSUM, then fold into the per-j SBUF accumulator on DVE. The
    # last piece produces `tot` and triggers the output chain.
    sA = {}
    ot_pairs = {}

    def attnv_piece(b, j, t0, t1, bob_s, defer=False, alt_psum=False):
        pool = u2 if alt_psum else u3
        opp = pool.tile([P, 129], F32, tag="u2" if alt_psum else "u3")
        for i in range(t0, t1):
            nc.tensor.matmul(opp, lhsT=eTs[b][:, i, ts(j, P)],
                             rhs=vsb[:, i, 0:129],
                             start=(i == t0), stop=(i == t1 - 1))
        if t0 == 0:
            s = sap.tile([P, 129], F32, tag="sA")
            nc.vector.tensor_copy(s, opp)
            sA[(b, j)] = s
        elif t1 < NT:
            nc.vector.tensor_add(sA[(b, j)], opp, sA[(b, j)])
        else:
            tot = aop.tile([P, 129], F32, tag="tot")
            nc.vector.tensor_add(tot, opp, sA.pop((b, j)))
            if defer:
                return tot
            out_chain(b, j, tot, bob_s)

    def out_chain(b, j, tot, bob_s, tail=False):
        # normalize AFTER the Wo projection (row scale commutes with
        # matmul): the reciprocal runs concurrently with transpose+Wo
        ao = aop.tile([P, C], BF16, tag="ao")
        nc.gpsimd.tensor_copy(ao, tot[:, 0:128])
        r = smallp.tile([P, 1], F32, tag="r")
        nc.vector.reciprocal(r, tot[:, 128:129])
        if tail and j % 2 == 1:
            # tail: score PSUM banks are free; alternating pools doubles
            # the number of in-flight output chains
            tfp = u1.tile([P, 4, C], F32, tag="u1")
        else:
            tfp = u2.tile([P, 4, C], F32, tag="u2")
        nc.tensor.matmul(tfp[:, 0, :], lhsT=ao, rhs=id_b,
                         start=True, stop=True)
        aoT = aop.tile([P, C], BF16, tag="aoT")
        if tail:
            # the exp stream is over: ACT is free to evacuate PSUM, keeping
            # DVE down to the reciprocal and final scale
            nc.scalar.copy(aoT, tfp[:, 0, :])
        else:
            nc.vector.tensor_copy(aoT, tfp[:, 0, :])
        nc.tensor.matmul(tfp[:, 1, :], lhsT=aoT, rhs=w_b["wo"],
                         start=True, stop=True)
        if j % 2 == 0:
            ot_pair = outp.tile([P, 2, C], F32, tag="ot")
            ot_pairs[b] = ot_pair
        ot = ot_pairs[b]
        nc.vector.scalar_tensor_tensor(
            ot[:, j % 2, :], tfp[:, 1, :], r, bob_s,
            mybir.AluOpType.mult, mybir.AluOpType.add)
        if j % 2 == 1:
            # one DMA per chain pair (rows are adjacent): halves the
            # descriptor-generation serialization at the drain
            eng = nc.gpsimd if (tail and j == NJ - 3) else nc.sync
            eng.dma_start(
                out=out[ds(b * QBLK + (j - 1) * P, 2 * P), :]
                .rearrange("(i p) c -> p i c", p=P),
                in_=ot)

    # ---- schedule -------------------------------------------------------
    # Queries are x token-tiles 0..15 (the host rotates each core's batch so
    # its query half leads), so there is no separate xq pipeline: q blocks
    # project straight out of nxT once groups 0/1 are normalized.
    def hook0(half):
        # k, q-half, and the matching 512-col slice of score tile 0 all
        # emit inside this half's hook: the first exp fires off the h0
        # chain alone instead of waiting for the whole group
        kv_half(0, half, with_v=False, on_act=True)
        emit_qproj_half(0, half, on_act=True)
        sp = u3.tile([P, 512], F32, tag="u3")
        nc.tensor.matmul(sp, lhsT=kT[:, ts(0, P)],
                         rhs=qT[:, ds(half * 512, 512)],
                         start=True, stop=True)
        nc.scalar.activation(eTs[0][:, 0, ds(half * 512, 512)], sp,
                             func=mybir.ActivationFunctionType.Exp)

    ln_group0(xg_x0, 0, nxT, half_hook=hook0)
    bob_s = late_consts()

    states = {}

    # Everything else hangs off explicit post-score-tile emission points,
    # chosen so (a) each group's stats+rstd precede its applies by several
    # exp slots, (b) at most one attnv piece (one DVE PSUM-evacuation op)
    # lands per exp window, (c) a piece's exps are complete when PE reaches
    # it (PE runs ~2 exp slots ahead of ACT via the two score PSUM bufs).
    def post0(t):
        if t == 5:
            states[2] = ln_load(x, 2, nc.sync, nc.gpsimd)
        elif t == 9:
            states[3] = ln_load(x, 3, nc.sync, nc.gpsimd)
        elif t == 14:
            emit_qproj_half(1, 0)
        elif t == 16:
            emit_qproj_half(1, 1)
        if 10 <= t <= 17:
            attnv_piece(0, t - 10, 0, 8, bob_s)       # needs exp(0,7)
        elif 24 <= t <= 31:
            attnv_piece(0, t - 24, 8, 24, bob_s)      # reads exps (0,8..23)

    def hook(g, half):
        kv_half(g, half, with_v=False)
        # v halves land ahead of this hook's scores so attnv pieces
        # scheduled in later windows never read unwritten vsb tiles
        if g == 1 and half == 0:
            v_half(0, 0)
            v_half(0, 1)
        elif g == 1:
            v_half(1, 0)
            v_half(1, 1)
        else:
            v_half(g, half)
        # scores run one half behind kv production, so the kT tiles they
        # read were evacuated ~4.5us earlier and the per-half PE->DVE->PE
        # chain never gates the exp stream
        prev = g * 8 + 4 * half - 4
        for i in range(prev, prev + 4):
            emit_scores(0, i)
            post0(i)

    # group-1 load hoisted ahead of the score loop: its stats complete
    # before the exp stream starts, so the rstd pair runs in the idle
    # pre-stream ACT window instead of costing ~0.4us mid-stream
    states[1] = ln_load(x, 1, nc.sync, nc.gpsimd)
    post0(0)
    for i in range(1, 4):
        emit_scores(0, i)
        post0(i)
    for g in range(1, 4):
        ln_apply(states.pop(g), g, nxT,
                 half_hook=lambda half, g=g: hook(g, half))
    for i in range(28, 32):
        emit_scores(0, i)
        post0(i)

    def post1(i):
        if 3 <= i <= 10:
            attnv_piece(0, i - 3, 24, 32, bob_s)      # block-0 outputs
        elif 11 <= i <= 18:
            attnv_piece(1, i - 11, 0, 8, bob_s)       # needs exp(1,7)
        elif 19 <= i <= 26:
            attnv_piece(1, i - 19, 8, 16, bob_s)      # needs exp(1,15)
        if i >= 27:
            attnv_piece(1, i - 27, 16, 24, bob_s)     # needs exp(1,23)
            if i >= 29:
                attnv_piece(1, i - 24, 16, 24, bob_s)

    for i in range(NT):
        emit_scores(1, i)
        post1(i)
    # drain: all final pieces first (PE never stalls on a chain; psum slots
    # alternate u3/u2 so the DVE folds pipeline 4-deep), then the 8 output
    # chains, alternating ACT- and DVE-side PSUM evacuation
    tots = []
    for j in range(3):
        tots.append(attnv_piece(1, j, 24, 32, bob_s, defer=True,
                                alt_psum=(j % 2 == 1)))
    for j in range(NJ):
        if j + 3 < NJ:
            tots.append(attnv_piece(1, j + 3, 24, 32, bob_s, defer=True,
                                    alt_psum=(j % 2 == 1)))
        out_chain(1, j, tots[j], bob_s, tail=True)


_CACHE = {}


def _patch_act_tables():
    # Force every activation onto the natural_log_exp_and_others set (it has
    # both Ln and Exp; Copy/Identity are in every set). The default chooser
    # puts Ln and Exp in different sets, and LN interleaved with the softmax
    # exp stream then reloads tables (~2.7us) on every switch. Emptying the
    # other sets preserves dict order, so act_func_set_id indices stay
    # aligned with act_info.json.
    if getattr(bacc, "_act_tables_patched", False):
        return
    orig = bacc.get_activation_tables

    def patched(module_arch):
        tabs = orig(module_arch)
        keep = "natural_log_exp_and_others"
        if keep in tabs:
            tabs = {k: (v if k == keep else type(v)()) for k, v in tabs.items()}
        return tabs

    bacc.get_activation_tables = patched
    bacc._act_tables_patched = True


def _build():
    if "nc" in _CACHE:
        return _CACHE["nc"]
    _patch_act_tables()
    nc = bacc.Bacc("TRN2", target_bir_lowering=False, debug=False, num_devices=8)
    aps = {}
    for name, shape in (
        ("x", [S, C]),
        ("wq", [C, C]), ("wk", [C, C]), ("wv", [C, C]), ("wo", [C, C]),
        ("bq", [C, 1]), ("bob", [P, C]), ("ident", [P, P]),
    ):
        aps[name] = nc.dram_tensor(name, shape, F32, kind="ExternalInput").ap()
    aps["out"] = nc.dram_tensor("out", [SQ, C], F32, kind="ExternalOutput").ap()
    with tile.TileContext(nc) as tc:
        _attention_kernel(tc, aps)
    nc.compile()
    _CACHE["nc"] = nc
    return nc


def _host_fold(gamma, beta, Wq, bq, Wk, bk, Wv, bv, Wo, bo):
    scale = 1.0 / np.sqrt(np.float32(C))
    f = {}
    f["wq"] = (gamma[:, None] * Wq * scale).astype(np.float32)
    f["bq"] = ((beta @ Wq + bq) * scale).astype(np.float32).reshape(C, 1)
    f["wk"] = (gamma[:, None] * Wk).astype(np.float32)
    f["wv"] = (gamma[:, None] * Wv).astype(np.float32)
    # v bias (incl. beta@Wv) passes through softmax untouched; fold via Wo.
    bvf = (beta @ Wv + bv).astype(np.float32)
    f["wo"] = np.asarray(Wo, dtype=np.float32)
    bof = (np.asarray(bo, np.float32) + bvf @ np.asarray(Wo, np.float32))
    f["bob"] = np.ascontiguousarray(np.broadcast_to(bof, (P, C)))
    f["ident"] = np.eye(P, dtype=np.float32)
    return f


def make_in_maps(x, gamma, beta, Wq, bq, Wk, bk, Wv, bv, Wo, bo):
    x = np.asarray(x, dtype=np.float32)
    folded = _host_fold(
        np.asarray(gamma, np.float32), np.asarray(beta, np.float32),
        np.asarray(Wq, np.float32), np.asarray(bq, np.float32),
        np.asarray(Wk, np.float32), np.asarray(bk, np.float32),
        np.asarray(Wv, np.float32), np.asarray(bv, np.float32),
        np.asarray(Wo, np.float32), np.asarray(bo, np.float32))
    xs = x.reshape(B, S, C)
    in_maps = []
    for core in range(8):
        bi, half = core // 2, core % 2
        m = dict(folded)
        # rotate so this core's query half leads; attention is invariant
        # to k/v token order, and outputs only cover the query half
        m["x"] = np.ascontiguousarray(
            np.roll(xs[bi], -half * SQ, axis=0))
        in_maps.append(m)
    return in_maps


def assemble(results):
    full = np.empty((B, S, C), dtype=np.float32)
    for core in range(8):
        bi, half = core // 2, core % 2
        full[bi, half * SQ:(half + 1) * SQ] = results[core]["out"]
    return full.reshape(B, HH, WW, C)


def kernel(x, gamma, beta, Wq, bq, Wk, bk, Wv, bv, Wo, bo):
    nc = _build()
    in_maps = make_in_maps(x, gamma, beta, Wq, bq, Wk, bk, Wv, bv, Wo, bo)
    res = run_bass_kernel_spmd(nc, in_maps, list(range(8)))
    return assemble(res.results)
